# Initial kernel scaffold
#
"""Multi-head causal attention (S=2048, H=5120, 40 heads) on 8 trn2 cores.

Tensor-parallel over heads: each core computes QKV + attention for 5 heads,
then a partial o_proj contribution (over its 640 input features) for the FULL
output; the host sums the 8 partials and transposes.

Per-core layout:
  stage A: SBUF-accumulated over h-chunks of 512 so hsT and Wt stream once:
             qk^T[f, s]  (f on partitions)  f-tiles: 5 Q heads then 5 K heads
             V[s, f]     (s on partitions)  computed directly in the natural
                                            orientation (hsT tile stationary)
  stage C: per head, per q-chunk of 512:
             scores^T[k, q] = K^T_tile.T @ Q^T   (one matmul per k-tile)
             probs = exp(scale * scores^T) on ACT (no max-subtraction needed:
                     scores are O(5) so exp is safe in fp32)
             causal mask on diagonal k-tiles via DVE multiply
             rowsum via ones-vector matmul accumulated across k-tiles on PE
             out_head^T[d, q] = sum_t V_tile.T-contract probs^T on PE
             normalize with reciprocal rowsum broadcast (outer-product matmul)
  stage D: outT_partial[o, s] = o_projT_slice.T @ attn^T accumulated over the
           5 local feature tiles in PSUM, evacuated via ScalarE.

All matmuls run as float32r (fp32 data, FP22 multiply, fp32 PSUM accum):
full PE rate at moving-dim >= 256, ~1e-4 output rel err.
"""

import numpy as np

S = 2048
H = 5120
NH = 40
DH = 128
NCORES = 8
NH_LOC = NH // NCORES      # 5 heads per core
HIN = NH_LOC * DH          # 640
FTOT = 3 * NH_LOC          # 15 f-tiles of 128 (5 Q, 5 K, 5 V)
SCALE = float(DH) ** -0.5

_PROGRAM = None


def _build_program(reps=1, stages="ACD"):
    from contextlib import ExitStack

    import concourse.bacc as bacc
    import concourse.mybir as mybir
    import concourse.tile as tile

    f32 = mybir.dt.float32
    f32r = mybir.dt.float32r
    Exp = mybir.ActivationFunctionType.Exp

    nc = bacc.Bacc("TRN2", target_bir_lowering=False)
    hsT = nc.dram_tensor("hsT", [H, S], f32r, kind="ExternalInput")
    wt = nc.dram_tensor("wt", [H, FTOT * 128], f32r, kind="ExternalInput")
    opjt = nc.dram_tensor("opjt", [H // 128, 128, HIN], f32r, kind="ExternalInput")
    masks = nc.dram_tensor("masks", [128, 2048], f32, kind="ExternalInput")
    outT = nc.dram_tensor("outT", [H, S], f32, kind="ExternalOutput")

    with (
        nc.allow_low_precision(reason="fp32r pipeline: FP22 rounding ~6e-5 rel"),
        tile.TileContext(nc) as tc,
    ):
      for _rep in range(reps):
       with ExitStack() as ctx:
        persist = ctx.enter_context(tc.tile_pool(name="persist", bufs=1))
        qkT = persist.tile([128, 10 * S], f32r)        # Q^T,K^T per head (80 KB/p)
        vnat = persist.tile([128, 16 * HIN], f32r)     # V natural, 16 s-tiles (40 KB/p)
        ones_f = persist.tile([128, 129], f32)
        ones = persist.tile([128, 129], f32r)
        nc.vector.memset(ones_f, 1.0)
        nc.vector.tensor_copy(ones, ones_f)
        ones_col = ones[:, 0:1]
        ones_row = ones[0:1, 0:128]

        # ---- stage A: qk^T and V accumulation over h-chunks of 512 ----
        with (
            tc.tile_pool(name="ahs", bufs=5) as ahs,
            tc.tile_pool(name="awt", bufs=5) as awt,
            tc.tile_pool(name="psA", bufs=4, space="PSUM") as psA,
            tc.tile_pool(name="psV", bufs=4, space="PSUM") as psV,
        ):
            for hc in range(H // 512):
                hsts = []
                wtts = []
                for g in range(4):
                    hst = ahs.tile([128, S], f32r, tag="hs")
                    wtt = awt.tile([128, FTOT * 128], f32r, tag="wt")
                    h0 = hc * 512 + g * 128
                    # split loads so early matmuls (low sc / low f) start
                    # before the whole panel lands
                    nc.sync.dma_start(out=hst[:, 0:1024], in_=hsT[h0:h0 + 128, 0:1024])
                    nc.sync.dma_start(out=hst[:, 1024:2048], in_=hsT[h0:h0 + 128, 1024:2048])
                    nc.sync.dma_start(out=wtt[:, 0:960], in_=wt[h0:h0 + 128, 0:960])
                    nc.sync.dma_start(out=wtt[:, 960:1920], in_=wt[h0:h0 + 128, 960:1920])
                    hsts.append(hst)
                    wtts.append(wtt)
                # Q^T / K^T part: f on partitions
                for f in range(10):
                    for sc in range(4):
                        ps = psA.tile([128, 512], f32)
                        for g in range(4):
                            nc.tensor.matmul(
                                ps,
                                lhsT=wtts[g][:, f * 128:(f + 1) * 128],
                                rhs=hsts[g][:, sc * 512:(sc + 1) * 512],
                                start=(g == 0),
                                stop=(g == 3),
                            )
                        tgt = qkT[:, f * S + sc * 512: f * S + (sc + 1) * 512]
                        if hc == 0:
                            nc.vector.tensor_copy(tgt, ps)
                        else:
                            nc.vector.tensor_add(tgt, tgt, ps)
                # V part: s on partitions, natural orientation
                for st in range(16):
                    for half in range(2):
                        ps = psV.tile([128, 320], f32)
                        c0 = 10 * 128 + half * 320
                        for g in range(4):
                            nc.tensor.matmul(
                                ps,
                                lhsT=hsts[g][:, st * 128:(st + 1) * 128],
                                rhs=wtts[g][:, c0:c0 + 320],
                                start=(g == 0),
                                stop=(g == 3),
                            )
                        tgt = vnat[:, st * HIN + half * 320: st * HIN + (half + 1) * 320]
                        if hc == 0:
                            nc.vector.tensor_copy(tgt, ps)
                        else:
                            nc.vector.tensor_add(tgt, tgt, ps)

        if "C" not in stages:
            # DCE guard: stream accumulated tensors out so stage A survives
            nc.sync.dma_start(
                out=outT[0:1280, :].bitcast(f32r).rearrange("(a p) s -> p a s", p=128),
                in_=qkT.rearrange("p (a s) -> p a s", a=10),
            )
            nc.sync.dma_start(
                out=outT[1280:1920, :].bitcast(f32r).rearrange("(a p) s -> p a s", p=128),
                in_=vnat.rearrange("p (a s) -> p a s", a=5),
            )
            continue
        # ---- stage C: attention per head ----
        catt = ctx.enter_context(tc.tile_pool(name="catt", bufs=1))
        attn = catt.tile([128, NH_LOC * S], f32r)      # attn^T per head (40 KB/p)
        with (
            tc.tile_pool(name="cmask", bufs=1) as cmask,
            tc.tile_pool(name="cP", bufs=1) as cP,
            tc.tile_pool(name="crecip", bufs=2) as crecip,
            tc.tile_pool(name="cb", bufs=1) as cb,
            tc.tile_pool(name="psCs", bufs=2, space="PSUM") as psCs,
            tc.tile_pool(name="psCo", bufs=2, space="PSUM") as psCo,
            tc.tile_pool(name="psCr", bufs=1, space="PSUM") as psCr,
            tc.tile_pool(name="psCb", bufs=1, space="PSUM") as psCb,
        ):
            masks_sb = cmask.tile([128, 2048], f32)
            nc.sync.dma_start(out=masks_sb, in_=masks[:, :])
            for h in range(NH_LOC):
                qoff = h * S
                koff = (NH_LOC + h) * S
                for j in range(4):
                    T = 4 * (j + 1)
                    # natural order: rowsum/PV consume t=0 first and the
                    # masked diagonal tiles (t>=4j) last, so each tile's
                    # exp+mask latency hides behind earlier consumers
                    order = list(range(T))
                    P_sb = cP.tile([128, 16 * 512], f32r, tag="P")
                    # scores^T tiles + exp (grouped by 2 psum banks) + mask
                    for gi in range(T // 2):
                        pair = order[2 * gi: 2 * gi + 2]
                        ps_s = psCs.tile([128, 1024], f32)
                        for w, t in enumerate(pair):
                            nc.tensor.matmul(
                                ps_s[:, w * 512:(w + 1) * 512],
                                lhsT=qkT[:, koff + t * 128: koff + (t + 1) * 128],
                                rhs=qkT[:, qoff + j * 512: qoff + (j + 1) * 512],
                                start=True,
                                stop=True,
                            )
                        if pair[1] == pair[0] + 1 and (pair[0] * 512) % 1024 == 0:
                            # adjacent destination: one wide exp
                            nc.scalar.activation(
                                P_sb[:, pair[0] * 512: pair[0] * 512 + 1024],
                                ps_s, Exp, scale=SCALE,
                            )
                        else:
                            for w, t in enumerate(pair):
                                nc.scalar.activation(
                                    P_sb[:, t * 512:(t + 1) * 512],
                                    ps_s[:, w * 512:(w + 1) * 512], Exp, scale=SCALE,
                                )
                        for w, t in enumerate(pair):
                            if t >= 4 * j:
                                r = t - 4 * j
                                nc.vector.tensor_mul(
                                    P_sb[:, t * 512:(t + 1) * 512],
                                    P_sb[:, t * 512:(t + 1) * 512],
                                    masks_sb[:, r * 512:(r + 1) * 512],
                                )
                    # rowsum over k via ones-vector matmuls accumulated on PE
                    # (non-diagonal tiles first: diagonal masks finish late)
                    rs_order = list(range(0, 4 * j)) + list(range(4 * j, T))
                    ps_r = psCr.tile([1, 512], f32)
                    for i, t in enumerate(rs_order):
                        nc.tensor.matmul(
                            ps_r,
                            lhsT=ones_col,
                            rhs=P_sb[:, t * 512:(t + 1) * 512],
                            start=(i == 0),
                            stop=(i == T - 1),
                        )
                    recip = crecip.tile([1, 512], f32r)
                    nc.vector.reciprocal(recip, ps_r)
                    # PV accumulation
                    ps_o = psCo.tile([128, 512], f32)
                    for t in range(T):
                        nc.tensor.matmul(
                            ps_o,
                            lhsT=vnat[:, t * HIN + h * 128: t * HIN + (h + 1) * 128],
                            rhs=P_sb[:, t * 512:(t + 1) * 512],
                            start=(t == 0),
                            stop=(t == T - 1),
                        )
                    # broadcast reciprocal over partitions via outer product
                    ps_b = psCb.tile([128, 512], f32)
                    nc.tensor.matmul(
                        ps_b,
                        lhsT=ones_row,
                        rhs=recip,
                        start=True,
                        stop=True,
                    )
                    bcast = cb.tile([128, 512], f32)
                    nc.vector.tensor_copy(bcast, ps_b)
                    nc.vector.tensor_mul(
                        attn[:, h * S + j * 512: h * S + (j + 1) * 512], ps_o, bcast
                    )

        if "D" not in stages:
            nc.sync.dma_start(
                out=outT[0:640, :].bitcast(f32r).rearrange("(a p) s -> p a s", p=128),
                in_=attn.rearrange("p (a s) -> p a s", a=5),
            )
            continue
        # ---- stage D: partial o_proj over local features ----
        with (
            tc.tile_pool(name="dop", bufs=4) as dop,
            tc.tile_pool(name="dout", bufs=4) as dout,
            tc.tile_pool(name="psD", bufs=4, space="PSUM") as psD,
        ):
            for ot in range(H // 128):
                opt_t = dop.tile([128, NH_LOC * 128], f32r)
                nc.sync.dma_start(out=opt_t, in_=opjt[ot, :, :])
                for sc in range(4):
                    ps = psD.tile([128, 512], f32)
                    for hi in range(NH_LOC):
                        nc.tensor.matmul(
                            ps,
                            lhsT=opt_t[:, hi * 128:(hi + 1) * 128],
                            rhs=attn[:, hi * S + sc * 512: hi * S + (sc + 1) * 512],
                            start=(hi == 0),
                            stop=(hi == NH_LOC - 1),
                        )
                    ob = dout.tile([128, 512], f32)
                    nc.scalar.copy(ob, ps)
                    nc.sync.dma_start(
                        out=outT[ot * 128:(ot + 1) * 128, sc * 512:(sc + 1) * 512],
                        in_=ob,
                    )

    nc.compile()
    return nc


def _get_program():
    global _PROGRAM
    if _PROGRAM is None:
        _PROGRAM = _build_program()
    return _PROGRAM


def _make_masks():
    m = np.zeros((128, 2048), np.float32)
    kk = np.arange(128)[:, None]
    th = np.arange(512)[None, :]
    for r in range(4):
        m[:, r * 512:(r + 1) * 512] = (th >= 128 * r + kk).astype(np.float32)
    return m


def make_in_maps(hidden_states, W_pack, o_proj):
    hsT = np.ascontiguousarray(hidden_states.T)
    masks = _make_masks()
    in_maps = []
    for i in range(NCORES):
        lo, hi = HIN * i, HIN * (i + 1)
        wq = W_pack[lo:hi]
        wk = W_pack[H + lo: H + hi]
        wv = W_pack[2 * H + lo: 2 * H + hi]
        wt_i = np.ascontiguousarray(np.concatenate([wq, wk, wv], axis=0).T)
        # [40, 128, 640]: opjt_t[ot, p, g*128+n] = o_proj[ot*128+n, lo+g*128+p]
        x = o_proj[:, lo:hi].T.reshape(NH_LOC, 128, H // 128, 128)
        opjt_i = np.ascontiguousarray(x.transpose(2, 1, 0, 3).reshape(H // 128, 128, HIN))
        in_maps.append({"hsT": hsT, "wt": wt_i, "opjt": opjt_i, "masks": masks})
    return in_maps


def kernel(hidden_states, W_pack, o_proj):
    from concourse.bass_utils import run_bass_kernel_spmd

    nc = _get_program()
    in_maps = make_in_maps(hidden_states, W_pack, o_proj)
    res = run_bass_kernel_spmd(nc, in_maps, core_ids=list(range(NCORES)))
    acc = res.results[0]["outT"].copy()
    for r in res.results[1:]:
        acc += r["outT"]
    return np.ascontiguousarray(acc.T)



# revision 7
# speedup vs baseline: 1.0842x; 1.0842x over previous
"""Multi-head causal attention (S=2048, H=5120, 40 heads) on 8 trn2 cores.

Tensor-parallel over heads: each core computes QKV + attention for 5 heads,
then a partial o_proj contribution (over its 640 input features) for the FULL
output; the host sums the 8 partials and transposes.

All operands are pre-cast to bf16 on the host (halves HBM traffic; bf16
matmuls run at the same 1 cycle/row PE rate as fp32r) and pre-swizzled so
every DMA is contiguous.

Per-core layout:
  stage A: QKV projection with the 5120-deep contraction split into 2 passes
           of 20 h-tiles each.  Within a pass each output tile accumulates
           its 20 matmuls entirely in PSUM (no DVE accumulation); pass 1
           evacuates via ScalarE copy (f32 PSUM -> bf16 SBUF), pass 2 adds
           into the same SBUF tile via one DVE tensor_add per tile.
             qk^T[f, s]  (f on partitions)  f-tiles: 5 Q heads then 5 K heads
             V[s, f]     (s on partitions)  natural orientation for PV
  stage C: per head, per q-chunk of 512:
             scores^T[k, q] = K^T_tile.T @ Q^T   (one matmul per k-tile)
             P = exp(scale * scores^T) on ACT -> bf16 (scores are O(5), no
                 max-subtraction needed)
             causal mask on diagonal k-tiles via DVE multiply (bf16 2x mode)
             rowsum via ones-vector matmul accumulated across k-tiles on PE
             out_head^T[d, q] = sum_t V_tile.T-contract probs^T on PE
             normalize with reciprocal rowsum broadcast (outer-product matmul)
  stage D: outT_partial[o, s] = o_projT_slice.T @ attn^T accumulated over the
           5 local feature tiles in PSUM, staged per ot-row in SBUF (bf16)
           and written with one DMA per 128-row block.

Host sums the 8 bf16 partials in fp32 and transposes.
"""

import numpy as np

S = 2048
H = 5120
NH = 40
DH = 128
NCORES = 8
NH_LOC = NH // NCORES      # 5 heads per core
HIN = NH_LOC * DH          # 640
NF_QK = 2 * NH_LOC         # 10 f-tiles of 128 (5 Q then 5 K)
HC = H // 128              # 40 h-tiles of 128
HALF = HC // 2             # 20 h-tiles per contraction pass
SCALE = float(DH) ** -0.5

_PROGRAM = None


def _build_program(reps=1, stages="ACD"):
    from contextlib import ExitStack

    import concourse.bacc as bacc
    import concourse.mybir as mybir
    import concourse.tile as tile

    f32 = mybir.dt.float32
    f32r = mybir.dt.float32r
    bf16 = mybir.dt.bfloat16
    Exp = mybir.ActivationFunctionType.Exp

    nc = bacc.Bacc("TRN2", target_bir_lowering=False)
    # hsT_d[p, hc*S + s] = hs[s, hc*128 + p]
    hsT_d = nc.dram_tensor("hsT", [128, HC * S], bf16, kind="ExternalInput")
    # wqk_d[f, p, hc*128 + c] = W_pack[row(f) + c, hc*128 + p]
    wqk_d = nc.dram_tensor("wqk", [NF_QK, 128, HC * 128], bf16, kind="ExternalInput")
    # wv_d[g, p, hc*320 + c] = W_pack[2H + lo + g*320 + c, hc*128 + p]
    wv_d = nc.dram_tensor("wv", [2, 128, HC * 320], bf16, kind="ExternalInput")
    # opjt_d[ot, p, hi*128 + n] = o_proj[ot*128 + n, lo + hi*128 + p]
    opjt_d = nc.dram_tensor("opjt", [H // 128, 128, HIN], bf16, kind="ExternalInput")
    masks_d = nc.dram_tensor("masks", [128, 2048], bf16, kind="ExternalInput")
    outT = nc.dram_tensor("outT", [H, S], bf16, kind="ExternalOutput")

    with (
        nc.allow_low_precision(reason="bf16 pipeline: ~5e-3 scale-relative"),
        tile.TileContext(nc) as tc,
    ):
      for _rep in range(reps):
       with ExitStack() as ctx:
        persist = ctx.enter_context(tc.tile_pool(name="persist", bufs=1))
        qkT = persist.tile([128, NF_QK * S], bf16)     # Q^T,K^T per head (40 KB/p)
        vnat = persist.tile([128, 16 * HIN], bf16)     # V natural, 16 s-tiles (20 KB/p)
        ones_f = persist.tile([128, 129], f32)
        ones_b = persist.tile([128, 129], bf16)
        ones_r = persist.tile([1, 129], f32r)
        nc.vector.memset(ones_f, 1.0)
        nc.vector.tensor_copy(ones_b, ones_f)
        nc.vector.tensor_copy(ones_r, ones_f[0:1, :])
        ones_col = ones_b[:, 0:1]
        ones_row = ones_r[:, 0:128]

        # ---- stage A: QKV with PSUM-resident contraction chains ----
        with (
            tc.tile_pool(name="ahs", bufs=2) as ahs,
            tc.tile_pool(name="awqk", bufs=2) as awqk,
            tc.tile_pool(name="awv", bufs=2) as awv,
            tc.tile_pool(name="psQK", bufs=4, space="PSUM") as psQK,
            tc.tile_pool(name="psV", bufs=4, space="PSUM") as psV,
        ):
            for p_ in range(2):
                # two hsT quarters of 10 h-tiles each, DMA'd in 5-hc chunks
                quarters = []
                for qtr in range(2):
                    qt = ahs.tile([128, 10 * S], bf16, tag="hs")
                    base = (p_ * HALF + qtr * 10) * S
                    for ch in range(5):
                        nc.sync.dma_start(
                            out=qt[:, ch * 2 * S:(ch + 1) * 2 * S],
                            in_=hsT_d[:, base + ch * 2 * S: base + (ch + 1) * 2 * S],
                        )
                    quarters.append(qt)

                def hs_rhs(k, c0, c1):
                    qt = quarters[k // 10]
                    off = (k % 10) * S
                    return qt[:, off + c0: off + c1]

                # Q^T / K^T: f on partitions
                for f in range(NF_QK):
                    wt = awqk.tile([128, HALF * 128], bf16, tag="wqk")
                    nc.sync.dma_start(
                        out=wt,
                        in_=wqk_d[f, :, p_ * HALF * 128:(p_ + 1) * HALF * 128],
                    )
                    for sc in range(4):
                        ps = psQK.tile([128, 512], f32)
                        for k in range(HALF):
                            nc.tensor.matmul(
                                ps,
                                lhsT=wt[:, k * 128:(k + 1) * 128],
                                rhs=hs_rhs(k, sc * 512, (sc + 1) * 512),
                                start=(k == 0),
                                stop=(k == HALF - 1),
                            )
                        tgt = qkT[:, f * S + sc * 512: f * S + (sc + 1) * 512]
                        if p_ == 0:
                            nc.scalar.copy(tgt, ps)
                        else:
                            nc.vector.tensor_add(tgt, tgt, ps)
                # V: s on partitions, natural orientation
                for g in range(2):
                    wv_t = awv.tile([128, HALF * 320], bf16, tag="wv")
                    nc.sync.dma_start(
                        out=wv_t,
                        in_=wv_d[g, :, p_ * HALF * 320:(p_ + 1) * HALF * 320],
                    )
                    for st in range(16):
                        ps = psV.tile([128, 320], f32)
                        for k in range(HALF):
                            nc.tensor.matmul(
                                ps,
                                lhsT=hs_rhs(k, st * 128, (st + 1) * 128),
                                rhs=wv_t[:, k * 320:(k + 1) * 320],
                                start=(k == 0),
                                stop=(k == HALF - 1),
                            )
                        tgt = vnat[:, st * HIN + g * 320: st * HIN + (g + 1) * 320]
                        if p_ == 0:
                            nc.scalar.copy(tgt, ps)
                        else:
                            nc.vector.tensor_add(tgt, tgt, ps)

        if "C" not in stages:
            # DCE guard: stream accumulated tensors out so stage A survives
            nc.sync.dma_start(
                out=outT[0:1280, :].rearrange("(a p) s -> p a s", p=128),
                in_=qkT.rearrange("p (a s) -> p a s", a=10),
            )
            nc.sync.dma_start(
                out=outT[1280:1920, :].rearrange("(a p) s -> p a s", p=128),
                in_=vnat.rearrange("p (a s) -> p a s", a=5),
            )
            continue
        # ---- stage C: attention per head ----
        catt = ctx.enter_context(tc.tile_pool(name="catt", bufs=1))
        attn = catt.tile([128, NH_LOC * S], bf16)      # attn^T per head (20 KB/p)
        with (
            tc.tile_pool(name="cmask", bufs=1) as cmask,
            tc.tile_pool(name="cP", bufs=2) as cP,
            tc.tile_pool(name="crecip", bufs=2) as crecip,
            tc.tile_pool(name="cb", bufs=2) as cb,
            tc.tile_pool(name="psCs", bufs=2, space="PSUM") as psCs,
            tc.tile_pool(name="psCo", bufs=2, space="PSUM") as psCo,
            tc.tile_pool(name="psCr", bufs=1, space="PSUM") as psCr,
            tc.tile_pool(name="psCb", bufs=1, space="PSUM") as psCb,
        ):
            masks_sb = cmask.tile([128, 2048], bf16)
            nc.sync.dma_start(out=masks_sb, in_=masks_d[:, :])
            for h in range(NH_LOC):
                qoff = h * S
                koff = (NH_LOC + h) * S
                for j in range(4):
                    T = 4 * (j + 1)
                    P_sb = cP.tile([128, 16 * 512], bf16, tag="P")
                    # scores^T tiles + exp (grouped by 2 psum banks) + mask
                    for gi in range(T // 2):
                        t0 = 2 * gi
                        ps_s = psCs.tile([128, 1024], f32)
                        for w in range(2):
                            t = t0 + w
                            nc.tensor.matmul(
                                ps_s[:, w * 512:(w + 1) * 512],
                                lhsT=qkT[:, koff + t * 128: koff + (t + 1) * 128],
                                rhs=qkT[:, qoff + j * 512: qoff + (j + 1) * 512],
                                start=True,
                                stop=True,
                            )
                        # adjacent destination: one wide exp -> bf16 P
                        nc.scalar.activation(
                            P_sb[:, t0 * 512: t0 * 512 + 1024],
                            ps_s, Exp, scale=SCALE,
                        )
                        for w in range(2):
                            t = t0 + w
                            if t >= 4 * j:
                                r = t - 4 * j
                                nc.vector.tensor_mul(
                                    P_sb[:, t * 512:(t + 1) * 512],
                                    P_sb[:, t * 512:(t + 1) * 512],
                                    masks_sb[:, r * 512:(r + 1) * 512],
                                )
                    # rowsum over k via ones-vector matmuls accumulated on PE
                    # (non-diagonal tiles first: diagonal masks finish late)
                    rs_order = list(range(0, 4 * j)) + list(range(4 * j, T))
                    ps_r = psCr.tile([1, 512], f32)
                    for i, t in enumerate(rs_order):
                        nc.tensor.matmul(
                            ps_r,
                            lhsT=ones_col,
                            rhs=P_sb[:, t * 512:(t + 1) * 512],
                            start=(i == 0),
                            stop=(i == T - 1),
                        )
                    recip = crecip.tile([1, 512], f32r)
                    nc.vector.reciprocal(recip, ps_r)
                    # PV accumulation
                    ps_o = psCo.tile([128, 512], f32)
                    for t in range(T):
                        nc.tensor.matmul(
                            ps_o,
                            lhsT=vnat[:, t * HIN + h * 128: t * HIN + (h + 1) * 128],
                            rhs=P_sb[:, t * 512:(t + 1) * 512],
                            start=(t == 0),
                            stop=(t == T - 1),
                        )
                    # broadcast reciprocal over partitions via outer product
                    ps_b = psCb.tile([128, 512], f32)
                    nc.tensor.matmul(
                        ps_b,
                        lhsT=ones_row,
                        rhs=recip,
                        start=True,
                        stop=True,
                    )
                    bcast = cb.tile([128, 512], f32)
                    nc.vector.tensor_copy(bcast, ps_b)
                    nc.vector.tensor_mul(
                        attn[:, h * S + j * 512: h * S + (j + 1) * 512], ps_o, bcast
                    )

        if "D" not in stages:
            nc.sync.dma_start(
                out=outT[0:640, :].rearrange("(a p) s -> p a s", p=128),
                in_=attn.rearrange("p (a s) -> p a s", a=5),
            )
            continue
        # ---- stage D: partial o_proj over local features ----
        with (
            tc.tile_pool(name="dop", bufs=3) as dop,
            tc.tile_pool(name="dout", bufs=3) as dout,
            tc.tile_pool(name="psD", bufs=4, space="PSUM") as psD,
        ):
            for ot in range(H // 128):
                opt_t = dop.tile([128, HIN], bf16, tag="op")
                nc.sync.dma_start(out=opt_t, in_=opjt_d[ot, :, :])
                ob = dout.tile([128, 2048], bf16, tag="ob")
                for sc in range(4):
                    ps = psD.tile([128, 512], f32)
                    for hi in range(NH_LOC):
                        nc.tensor.matmul(
                            ps,
                            lhsT=opt_t[:, hi * 128:(hi + 1) * 128],
                            rhs=attn[:, hi * S + sc * 512: hi * S + (sc + 1) * 512],
                            start=(hi == 0),
                            stop=(hi == NH_LOC - 1),
                        )
                    nc.vector.tensor_copy(ob[:, sc * 512:(sc + 1) * 512], ps)
                nc.sync.dma_start(
                    out=outT[ot * 128:(ot + 1) * 128, :],
                    in_=ob,
                )

    nc.compile()
    return nc


def _get_program():
    global _PROGRAM
    if _PROGRAM is None:
        _PROGRAM = _build_program()
    return _PROGRAM


def _make_masks():
    m = np.zeros((128, 2048), np.float32)
    kk = np.arange(128)[:, None]
    th = np.arange(512)[None, :]
    for r in range(4):
        m[:, r * 512:(r + 1) * 512] = (th >= 128 * r + kk).astype(np.float32)
    return m


def _bf16(x):
    import ml_dtypes

    return np.ascontiguousarray(np.asarray(x, dtype=np.float32).astype(ml_dtypes.bfloat16))


def make_in_maps(hidden_states, W_pack, o_proj):
    hidden_states = np.asarray(hidden_states)
    W_pack = np.asarray(W_pack)
    o_proj = np.asarray(o_proj)
    # hsT_d[p, hc*S + s] = hs[s, hc*128 + p]
    hsT = _bf16(
        hidden_states.T.reshape(HC, 128, S).transpose(1, 0, 2).reshape(128, HC * S)
    )
    masks = _bf16(_make_masks())
    in_maps = []
    for i in range(NCORES):
        lo, hi = HIN * i, HIN * (i + 1)
        # Q then K feature tiles: rows of W_pack
        wqk = np.empty((NF_QK, 128, HC * 128), np.float32)
        for fi in range(NF_QK):
            row0 = (lo if fi < NH_LOC else H + lo - NH_LOC * 128) + fi * 128
            blk = W_pack[row0:row0 + 128, :]           # [128 f, 5120 h]
            wqk[fi] = blk.T.reshape(HC, 128, 128).transpose(1, 0, 2).reshape(128, HC * 128)
        wv = np.empty((2, 128, HC * 320), np.float32)
        for g in range(2):
            blk = W_pack[2 * H + lo + g * 320: 2 * H + lo + (g + 1) * 320, :]  # [320 f, H]
            wv[g] = blk.T.reshape(HC, 128, 320).transpose(1, 0, 2).reshape(128, HC * 320)
        # opjt_d[ot, p, hi*128 + n] = o_proj[ot*128 + n, lo + hi*128 + p]
        x = o_proj[:, lo:hi].T.reshape(NH_LOC, 128, H // 128, 128)
        opjt = x.transpose(2, 1, 0, 3).reshape(H // 128, 128, HIN)
        in_maps.append({
            "hsT": hsT,
            "wqk": _bf16(wqk),
            "wv": _bf16(wv),
            "opjt": _bf16(opjt),
            "masks": masks,
        })
    return in_maps


def kernel(hidden_states, W_pack, o_proj):
    from concourse.bass_utils import run_bass_kernel_spmd

    nc = _get_program()
    in_maps = make_in_maps(hidden_states, W_pack, o_proj)
    res = run_bass_kernel_spmd(nc, in_maps, core_ids=list(range(NCORES)))
    acc = np.zeros((H, S), np.float32)
    for r in res.results:
        acc += np.asarray(r["outT"], dtype=np.float32)
    return np.ascontiguousarray(acc.T)
